# revision 6
# baseline (speedup 1.0000x reference)
"""Trainium2 kernel for nn_PatternsOfThinkingBlock (topk_masking).

reference:
  idx = argmax(x, -1); gathered = x[..., idx]
  y = gelu(einsum('bhs,ts->bht', gathered, W) + b)   (exact erf gelu)
  out = x with x[b,h,s,idx[b,h,s]] = y[b,h,s]

Strategy: pure data parallel over the 32 (b,h) slices -> 4 per core on 8
cores.  Per core each slice is streamed through SBUF in [128, 2*2048]
blocks (pass-through copy to the output + per-row max/argmax on the DVE),
the tiny matvec y = gelu(W @ gathered + b) runs on the PE against a
resident transposed copy of W, and the 2048 modified elements per slice
are written with one indirect (scatter) DMA.
"""

import numpy as np

import concourse.bacc as bacc
import concourse.bass as bass
import concourse.mybir as mybir
import concourse.tile as tile
from concourse import bass_utils

F32 = mybir.dt.float32
U32 = mybir.dt.uint32

S = 2048
NSL = 4            # bh slices per core
AR = 2             # 128-row chunks per streamed block
N_CORES = 8
C = S // 128


def _build(n_cores=N_CORES, repeat=1, internal_io=False):
    nblk = S // (128 * AR)
    nc = bacc.Bacc("TRN2", target_bir_lowering=False, debug=False,
                   num_devices=n_cores)

    # internal_io: timing-only variant — big tensors live in device DRAM
    # (no host staging per call), tiny external in/out keep the graph alive.
    big_in = "Internal" if internal_io else "ExternalInput"
    big_out = "Internal" if internal_io else "ExternalOutput"
    xs = nc.dram_tensor("xs", (NSL, S, S), F32, kind=big_in).ap()
    wt = nc.dram_tensor("wt", (S, S), F32, kind=big_in).ap()
    bias = nc.dram_tensor("bias", (S,), F32, kind=big_in).ap()
    iota2d = nc.dram_tensor("iota2d", (128, C), F32, kind=big_in).ap()
    outs = [nc.dram_tensor(f"out{n}", (S, S), F32, kind=big_out).ap()
            for n in range(NSL)]
    if internal_io:
        dum_in = nc.dram_tensor("dum_in", (128, 4), F32,
                                kind="ExternalInput").ap()
        dum_out = nc.dram_tensor("dum_out", (128, NSL + 4), F32,
                                 kind="ExternalOutput").ap()

    with tile.TileContext(nc) as tc:
        with tc.tile_pool(name="resident", bufs=1) as rpool, \
             tc.tile_pool(name="blocks", bufs=4) as bpool, \
             tc.tile_pool(name="psum", bufs=2, space="PSUM") as ppool, \
             tc.tile_pool(name="small", bufs=2) as spool:

            wt_sb = rpool.tile([128, C * S], F32)
            bias_sb = rpool.tile([128, C], F32)
            iota_sb = rpool.tile([128, C], F32)
            max_all = rpool.tile([128, NSL * C * 8], F32)
            idx_all = rpool.tile([128, NSL * C * 8], U32)
            y_all = rpool.tile([128, NSL * C], F32)
            off_all = rpool.tile([128, NSL * C], U32)

            for c in range(C):
                nc.sync.dma_start(wt_sb[:, c * S:(c + 1) * S],
                                  wt[c * 128:(c + 1) * 128, :])
            nc.sync.dma_start(bias_sb[:], bias.rearrange("(c p) -> p c", p=128))
            nc.sync.dma_start(iota_sb[:], iota2d[:, :])

            for n in range(NSL * repeat):
                n = n % NSL
                xn = xs[n]
                on = outs[n]
                for blk in range(nblk):
                    r0 = blk * 128 * AR
                    src = xn[r0:r0 + 128 * AR, :].rearrange(
                        "(a p) m -> p a m", p=128)
                    dst = on[r0:r0 + 128 * AR, :].rearrange(
                        "(a p) m -> p a m", p=128)
                    xt = bpool.tile([128, AR * S], F32, tag="xt")
                    xt3 = xt[:].rearrange("p (a m) -> p a m", a=AR)
                    nc.sync.dma_start(xt3, src)
                    nc.sync.dma_start(dst, xt3)
                    for a in range(AR):
                        rb = blk * AR + a
                        col = n * C + rb
                        sub = xt[:, a * S:(a + 1) * S]
                        nc.vector.max(max_all[:, col * 8:(col + 1) * 8], sub)
                        nc.vector.max_index(
                            idx_all[:, col * 8:(col + 1) * 8],
                            max_all[:, col * 8:(col + 1) * 8], sub)

                # y = gelu(W @ gathered + b)
                psum_t = ppool.tile([128, C], F32, tag="psum_y")
                for tci in range(C):
                    for sc in range(C):
                        nc.tensor.matmul(
                            psum_t[:, tci:tci + 1],
                            wt_sb[:, sc * S + tci * 128: sc * S + (tci + 1) * 128],
                            max_all[:, (n * C + sc) * 8:(n * C + sc) * 8 + 1],
                            start=(sc == 0), stop=(sc == C - 1))
                y_pre = spool.tile([128, C], F32, tag="ypre")
                nc.vector.tensor_add(y_pre[:], psum_t[:], bias_sb[:])
                nc.scalar.activation(y_all[:, n * C:(n + 1) * C], y_pre[:],
                                     mybir.ActivationFunctionType.Gelu)

                # element offsets within out{n}: off[p, rb] = p*S + rb*128*S + idx
                idx_view = idx_all[:, n * C * 8:(n + 1) * C * 8].rearrange(
                    "p (c e) -> p c e", e=8)[:, :, 0]
                nc.vector.tensor_tensor(
                    off_all[:, n * C:(n + 1) * C], idx_view, iota_sb[:],
                    op=mybir.AluOpType.add)

                # HW consumes one offset per partition and writes the whole
                # in_ row from it, so scatter one [128, 1] column per chunk.
                for rb in range(C):
                    col = n * C + rb
                    nc.gpsimd.indirect_dma_start(
                        out=on.rearrange("a (b o) -> (a b) o", o=1),
                        out_offset=bass.IndirectOffsetOnAxis(
                            ap=off_all[:, col:col + 1], axis=0),
                        in_=y_all[:, col:col + 1],
                        in_offset=None)

            if internal_io:
                # keep the outputs observably live + depend on the tiny input
                live = spool.tile([128, NSL + 4], F32, tag="live")
                nc.sync.dma_start(live[:, NSL:], dum_in[:])
                for n in range(NSL):
                    nc.sync.dma_start(live[:, n:n + 1], outs[n][:128, 0:1])
                nc.sync.dma_start(dum_out[:], live[:])

    nc.compile()
    return nc


_NC_CACHE = {}


def _get_nc():
    if "nc" not in _NC_CACHE:
        _NC_CACHE["nc"] = _build()
    return _NC_CACHE["nc"]


def _make_in_maps(x, W, b):
    x = np.ascontiguousarray(np.asarray(x, dtype=np.float32))
    W = np.asarray(W, dtype=np.float32)
    b = np.ascontiguousarray(np.asarray(b, dtype=np.float32))
    wt = np.ascontiguousarray(W.T)
    p = np.arange(128, dtype=np.float32)[:, None]
    c = np.arange(C, dtype=np.float32)[None, :]
    iota2d = np.ascontiguousarray(p * S + c * 128 * S)

    xf = x.reshape(-1, S, S)
    assert xf.shape[0] == N_CORES * NSL
    in_maps = []
    for core in range(N_CORES):
        in_maps.append({
            "xs": xf[core * NSL:(core + 1) * NSL],
            "wt": wt,
            "bias": b,
            "iota2d": iota2d,
        })
    return in_maps


def _run(in_maps, **kwargs):
    nc = _get_nc()
    return bass_utils.run_bass_kernel_spmd(
        nc, in_maps, core_ids=list(range(N_CORES)), **kwargs)


def kernel(x, W, b):
    shape = np.asarray(x).shape
    res = _run(_make_in_maps(x, W, b))
    parts = [res.results[core][f"out{n}"]
             for core in range(N_CORES) for n in range(NSL)]
    return np.stack(parts).reshape(shape)


# revision 11
# speedup vs baseline: 16.7006x; 16.7006x over previous
"""Trainium2 kernel for nn_PatternsOfThinkingBlock (topk_masking).

reference:
  idx = argmax(x, -1); gathered = x[..., idx]   (gathered == row max)
  y = gelu(einsum('bhs,ts->bht', gathered, W) + b)   (exact erf gelu)
  out = x with x[b,h,s,idx[b,h,s]] = y[b,h,s]

Strategy: pure data parallel over the 32 (b,h) slices -> 4 per core on 8
cores.  Per core, each [2048, 2048] slice is streamed once through SBUF
and kept resident (8 blocks of [128, 2*2048] f32).  While streaming in,
the DVE computes each row's max (== the gathered value) and the PE
accumulates the tiny matvec z = W @ gathered chunk-by-chunk against a
resident fp16 transposed copy of W.  At slice end y = gelu(z + b) and
delta = y - rowmax are formed, then each block is fixed up in SBUF
(out_row = x_row + (x_row == rowmax) * delta -- i.e. the argmax element
is replaced by y) and written out.  No DRAM scatter, single read +
single write of the big tensor.
"""

import numpy as np

import concourse.bacc as bacc
import concourse.bass as bass
import concourse.mybir as mybir
import concourse.tile as tile
from concourse import bass_utils

F32 = mybir.dt.float32
F16 = mybir.dt.float16

S = 2048
NSL = 4            # bh slices per core
N_CORES = 8
C = S // 128       # 16 row chunks per slice
NBLK = C // 2      # streamed blocks of 2 chunks


def _build(n_cores=N_CORES, repeat=1, internal_io=False):
    nc = bacc.Bacc("TRN2", target_bir_lowering=False, debug=False,
                   num_devices=n_cores)

    big_in = "Internal" if internal_io else "ExternalInput"
    big_out = "Internal" if internal_io else "ExternalOutput"
    xs = nc.dram_tensor("xs", (NSL, S, S), F32, kind=big_in).ap()
    wt = nc.dram_tensor("wt", (S, S), F16, kind=big_in).ap()
    bias = nc.dram_tensor("bias", (S,), F32, kind=big_in).ap()
    outs = [nc.dram_tensor(f"out{n}", (S, S), F32, kind=big_out).ap()
            for n in range(NSL)]
    if internal_io:
        dum_in = nc.dram_tensor("dum_in", (128, 4), F32,
                                kind="ExternalInput").ap()
        dum_out = nc.dram_tensor("dum_out", (128, NSL + 4), F32,
                                 kind="ExternalOutput").ap()

    with tile.TileContext(nc) as tc:
        with tc.tile_pool(name="resident", bufs=1) as rpool, \
             tc.tile_pool(name="blocks", bufs=NBLK) as bpool, \
             tc.tile_pool(name="tsc", bufs=1) as tpool, \
             tc.tile_pool(name="psum", bufs=2, space="PSUM") as ppool, \
             tc.tile_pool(name="small", bufs=2) as spool:

            wt_sb = rpool.tile([128, C * S], F16)
            bias_sb = rpool.tile([128, C], F32)
            gmax = rpool.tile([128, NSL * C], F32)   # row maxes (gathered)
            gh = rpool.tile([128, NSL * C], F16)     # fp16 copy for PE
            yv = rpool.tile([128, NSL * C], F32)
            dlt = rpool.tile([128, NSL * C], F32)
            # match_replace operand: col 0 of each 8-group = row max,
            # cols 1-7 stay at a value that never occurs in x
            mr_all = rpool.tile([128, C * 8], F32)

            nc.vector.memset(mr_all[:], -1e30)
            for c in range(C):
                nc.sync.dma_start(wt_sb[:, c * S:(c + 1) * S],
                                  wt[c * 128:(c + 1) * 128, :])
            nc.sync.dma_start(bias_sb[:], bias.rearrange("(c p) -> p c", p=128))

            for it in range(NSL * repeat):
                n = it % NSL
                xn = xs[n]
                on = outs[n]
                blocks = []
                # one-shot matmuls: chunk sc's 16 partial dots land in
                # psum columns [sc*16 .. sc*16+15]; summed over sc at the end
                psum_t = ppool.tile([128, C * C], F32, tag="psum_y")
                for blk in range(NBLK):
                    r0 = blk * 256
                    src = xn[r0:r0 + 256, :].rearrange("(a p) m -> p a m", p=128)
                    xt = bpool.tile([128, 2 * S], F32, tag="xt")
                    nc.sync.dma_start(xt[:].rearrange("p (a m) -> p a m", a=2),
                                      src)
                    blocks.append(xt)
                    for a in range(2):
                        sc = blk * 2 + a
                        col = n * C + sc
                        sub = xt[:, a * S:(a + 1) * S]
                        nc.vector.reduce_max(gmax[:, col:col + 1], sub,
                                             axis=mybir.AxisListType.X)
                        nc.vector.tensor_copy(gh[:, col:col + 1],
                                              gmax[:, col:col + 1])
                        nc.vector.tensor_copy(mr_all[:, sc * 8:sc * 8 + 1],
                                              gmax[:, col:col + 1])
                        for tci in range(C):
                            nc.tensor.matmul(
                                psum_t[:, sc * C + tci: sc * C + tci + 1],
                                wt_sb[:, sc * S + tci * 128:
                                      sc * S + (tci + 1) * 128],
                                gh[:, col:col + 1],
                                start=True, stop=True)

                ncol = slice(n * C, (n + 1) * C)
                y_pre = spool.tile([128, C], F32, tag="ypre")
                # z[tc] = sum_sc psum[:, sc*C + tc] via strided reduce
                psum_v = psum_t[:].rearrange("p (s t) -> p t s", s=C)
                nc.vector.reduce_sum(
                    y_pre[:].rearrange("p (t o) -> p t o", o=1), psum_v,
                    axis=mybir.AxisListType.X)
                nc.vector.tensor_add(y_pre[:], y_pre[:], bias_sb[:])
                nc.scalar.activation(yv[:, ncol], y_pre[:],
                                     mybir.ActivationFunctionType.Gelu)
                nc.vector.tensor_sub(dlt[:, ncol], yv[:, ncol], gmax[:, ncol])

                for blk in range(NBLK):
                    xt = blocks[blk]
                    for a in range(2):
                        sc = blk * 2 + a
                        col = n * C + sc
                        sub = xt[:, a * S:(a + 1) * S]
                        tsc = tpool.tile([128, S], F32, tag="tsc")
                        # first occurrence of the max -> sentinel (ties keep x)
                        nc.vector.match_replace(
                            tsc[:], in_to_replace=mr_all[:, sc * 8:(sc + 1) * 8],
                            in_values=sub, imm_value=1e30)
                        nc.vector.tensor_scalar(
                            tsc[:], tsc[:], 1e30, dlt[:, col:col + 1],
                            op0=mybir.AluOpType.is_equal,
                            op1=mybir.AluOpType.mult)
                        nc.any.tensor_add(sub, sub, tsc[:])
                    r0 = blk * 256
                    dst = on[r0:r0 + 256, :].rearrange("(a p) m -> p a m", p=128)
                    nc.sync.dma_start(dst,
                                      blocks[blk][:].rearrange(
                                          "p (a m) -> p a m", a=2))

            if internal_io:
                live = spool.tile([128, NSL + 4], F32, tag="live")
                nc.sync.dma_start(live[:, NSL:], dum_in[:])
                for n in range(NSL):
                    nc.sync.dma_start(live[:, n:n + 1], outs[n][:128, 0:1])
                nc.sync.dma_start(dum_out[:], live[:])

    nc.compile()
    return nc


_NC_CACHE = {}


def _get_nc():
    if "nc" not in _NC_CACHE:
        _NC_CACHE["nc"] = _build()
    return _NC_CACHE["nc"]


def _make_in_maps(x, W, b):
    x = np.ascontiguousarray(np.asarray(x, dtype=np.float32))
    W = np.asarray(W, dtype=np.float32)
    b = np.ascontiguousarray(np.asarray(b, dtype=np.float32))
    wt = np.ascontiguousarray(W.T.astype(np.float16))

    xf = x.reshape(-1, S, S)
    assert xf.shape[0] == N_CORES * NSL
    in_maps = []
    for core in range(N_CORES):
        in_maps.append({
            "xs": xf[core * NSL:(core + 1) * NSL],
            "wt": wt,
            "bias": b,
        })
    return in_maps


def _run(in_maps, **kwargs):
    nc = _get_nc()
    return bass_utils.run_bass_kernel_spmd(
        nc, in_maps, core_ids=list(range(N_CORES)), **kwargs)


def kernel(x, W, b):
    shape = np.asarray(x).shape
    res = _run(_make_in_maps(x, W, b))
    parts = [res.results[core][f"out{n}"]
             for core in range(N_CORES) for n in range(NSL)]
    return np.stack(parts).reshape(shape)
